# revision 12
# baseline (speedup 1.0000x reference)
"""MoE layer (8 experts, top-2) on 8 TRN2 NeuronCores — expert-parallel.

Each core c owns expert e=c. It routes ALL T=8192 tokens (fp32 router),
compacts the tokens assigned to its expert into a capacity-C buffer via an
indirect-scatter permutation, runs the expert FFN (fc1 -> gelu -> fc2) on the
compacted tokens, scales rows by the per-token combine weight, and returns
[C, H] compacted outputs plus the slot->token map. The host scatter-adds the
8 per-expert results into the full [T, H] output (the expert-parallel
"combine" step) and assembles the aux load-balance loss from per-expert
sums computed on device.

Self-contained: hardcodes all shapes for the nn_MoELayer problem
(B=4, S=2048, HIDDEN=1024, FFN=4096, E=8, TOP_K=2).
"""

import numpy as np

import concourse.bacc as bacc
import concourse.bass as bass
import concourse.mybir as mybir
import concourse.tile as tile
from concourse import bass_utils
from concourse.masks import make_identity, make_upper_triangular
from concourse.tile import add_dep_helper

P = 128
T = 8192          # tokens
H = 1024          # hidden
F = 4096          # ffn
E = 8             # experts
C = 2560          # per-expert capacity (observed max load 2175)
W = 1026          # xp row width: 1024 x + combine + token-id
XPN = C + T       # permutation buffer rows
NTT = T // P      # 64 token tiles
NTB = T // 512    # 16 router blocks
NS = C // P       # 20 slot tiles
NG = C // 512     # 5 fc1 groups of 512 slots
KH = H // P       # 8 contraction chunks over H
KF = F // P       # 32 contraction chunks over F

# FFN matmul input dtype: "float32" (exact, 4 cyc/row), "float32r"
# (fp32 storage, fast path, 1 cyc/row at N>=512), "bfloat16" (1 cyc/row).
FFN_DT = "float32r"
DEBUG = False
ABLATE = 0   # 0=full, 1=skip F2, 2=skip F1+F2, 3=skip T+F1+F2

f32 = mybir.dt.float32
i32 = mybir.dt.int32


def _round_f32r(a):
    """Replicate HW float32r rounding: RNE to 11 explicit mantissa bits."""
    x = np.ascontiguousarray(a, np.float32).view(np.uint32).astype(np.uint64)
    shift = 23 - 11
    add = (1 << (shift - 1)) - 1 + ((x >> shift) & 1)
    return ((x + add) >> shift << shift).astype(np.uint32).view(np.float32)


def _build_program():
    nc = bacc.Bacc("TRN2", target_bir_lowering=False, debug=False)

    ffn_store = mybir.dt.float32r if FFN_DT == "float32r" else f32

    # ---- I/O ----
    xTR = nc.dram_tensor("xTR", [P, NTB * KH * 512], f32, kind="ExternalInput")
    x_tm = nc.dram_tensor("x_tm", [T, H], f32, kind="ExternalInput")
    rwR = nc.dram_tensor("rwR", [P, KH * E], f32, kind="ExternalInput")
    w1R = nc.dram_tensor("w1R", [P, KF * KH * P], ffn_store, kind="ExternalInput")
    b1R = nc.dram_tensor("b1R", [P, KF], f32, kind="ExternalInput")
    w2R = nc.dram_tensor("w2R", [P, KF * H], ffn_store, kind="ExternalInput")
    b2 = nc.dram_tensor("b2", [1, H], f32, kind="ExternalInput")
    esel = nc.dram_tensor("esel", [P, E], f32, kind="ExternalInput")

    yc = nc.dram_tensor("yc", [C, H], f32, kind="ExternalOutput")
    if DEBUG:
        dbg_ids = nc.dram_tensor("dbg_ids", [P, NTT], f32, kind="ExternalOutput")
        dbg_slot = nc.dram_tensor("dbg_slot", [P, NTT], i32, kind="ExternalOutput")
        dbg_msk = nc.dram_tensor("dbg_msk", [P, NTT], f32, kind="ExternalOutput")
        dbg_cmb = nc.dram_tensor("dbg_cmb", [P, NTT], f32, kind="ExternalOutput")
        dbg_ld = nc.dram_tensor("dbg_ld", [P, W], f32, kind="ExternalOutput")
        dbg_posa = nc.dram_tensor("dbg_posa", [P, NTT], f32, kind="ExternalOutput")
        dbg_posu = nc.dram_tensor("dbg_posu", [P, NTT], f32, kind="ExternalOutput")
        dbg_slotf = nc.dram_tensor("dbg_slotf", [P, NTT], f32, kind="ExternalOutput")
        dbg_slot2 = nc.dram_tensor("dbg_slot2", [P, NTT], i32, kind="ExternalOutput")
        dbg_msku8 = nc.dram_tensor("dbg_msku8", [P, NTT], mybir.dt.uint8, kind="ExternalOutput")
    ids_out = nc.dram_tensor("ids_out", [P, NS], i32, kind="ExternalOutput")
    aux_out = nc.dram_tensor("aux_out", [E, 2], f32, kind="ExternalOutput")

    # ---- internal DRAM ----
    xp = nc.dram_tensor("xp", [XPN, W], f32)
    h_dram = nc.dram_tensor("h_dram", [P, KF * C], ffn_store)

    with tile.TileContext(nc) as tc:
        with tc.tile_pool(name="consts", bufs=1) as cp, \
             tc.tile_pool(name="route", bufs=1) as rp:

            ident = cp.tile([P, P], f32)
            make_identity(nc, ident[:])
            ustrict = cp.tile([P, P], f32)
            make_upper_triangular(nc, ustrict[:], val=1.0, diag=False)
            ones_col = cp.tile([P, 1], f32)
            nc.vector.memset(ones_col[:], 1.0)
            ones_row1 = cp.tile([1, P], f32)
            nc.vector.memset(ones_row1[:], 1.0)
            rw_sb = cp.tile([P, KH * E], f32)
            nc.sync.dma_start(out=rw_sb[:], in_=rwR[:, :])
            esel_sb = cp.tile([P, E], f32)
            nc.sync.dma_start(out=esel_sb[:], in_=esel[:, :])
            b1_sb = cp.tile([P, KF], f32)
            nc.sync.dma_start(out=b1_sb[:], in_=b1R[:, :])
            b2_sb = cp.tile([1, H], f32)
            nc.sync.dma_start(out=b2_sb[:], in_=b2[:, :])

            # persistent routing results
            cmb_cols = rp.tile([P, NTT], f32)     # combine weight, col=token tile
            msk_cols = rp.tile([P, NTT], f32)     # assignment mask
            ids_i32 = rp.tile([P, NTT], i32)
            nc.gpsimd.iota(ids_i32[:], pattern=[[P, NTT]], base=0,
                           channel_multiplier=1)
            ids_f32 = rp.tile([P, NTT], f32)
            nc.vector.tensor_copy(ids_f32[:], ids_i32[:])

            # ---------------- Phase R: routing ----------------
            with tc.tile_pool(name="psA", bufs=1, space="PSUM") as psA, \
                 tc.tile_pool(name="rblk", bufs=2) as rb, \
                 tc.tile_pool(name="rsm", bufs=3) as rs, \
                 tc.tile_pool(name="psR", bufs=3, space="PSUM") as psR:
                probs_sum_ps = psA.tile([E, 1], f32, space="PSUM")
                cnt_sum_ps = psA.tile([E, 1], f32, space="PSUM")
                for tb in range(NTB):
                    xTb = rb.tile([P, KH * 512], f32, tag="xTb")
                    nc.sync.dma_start(
                        out=xTb[:], in_=xTR[:, tb * KH * 512:(tb + 1) * KH * 512])
                    for m in range(4):
                        sub = tb * 4 + m
                        lg = psR.tile([P, E], f32, space="PSUM", tag="lg")
                        for k in range(KH):
                            nc.tensor.matmul(
                                lg[:],
                                lhsT=xTb[:, k * 512 + m * P: k * 512 + (m + 1) * P],
                                rhs=rw_sb[:, k * E:(k + 1) * E],
                                start=(k == 0), stop=(k == KH - 1))
                        # softmax pieces (selection uses unnormalized ex)
                        mx = rs.tile([P, 1], f32, tag="mx")
                        nc.vector.reduce_max(out=mx[:], in_=lg[:],
                                             axis=mybir.AxisListType.X)
                        mneg = rs.tile([P, 1], f32, tag="mneg")
                        nc.vector.tensor_scalar_mul(mneg[:], mx[:], -1.0)
                        ex = rs.tile([P, E], f32, tag="ex")
                        nc.scalar.activation(ex[:], lg[:],
                                             mybir.ActivationFunctionType.Exp,
                                             bias=mneg[:, 0:1], scale=1.0)
                        s = rs.tile([P, 1], f32, tag="s")
                        nc.vector.reduce_sum(out=s[:], in_=ex[:],
                                             axis=mybir.AxisListType.X)
                        r = rs.tile([P, 1], f32, tag="r")
                        nc.vector.reciprocal(r[:], s[:])
                        probs = rs.tile([P, E], f32, tag="probs")
                        nc.vector.tensor_scalar_mul(probs[:], ex[:], r[:, 0:1])
                        # top-2 of ex (sorted desc)
                        m8 = rs.tile([P, 8], f32, tag="m8")
                        nc.vector.max(out=m8[:], in_=ex[:])
                        den = rs.tile([P, 1], f32, tag="den")
                        nc.vector.tensor_add(den[:], m8[:, 0:1], m8[:, 1:2])
                        rd = rs.tile([P, 1], f32, tag="rd")
                        nc.vector.reciprocal(rd[:], den[:])
                        w1c = rs.tile([P, 1], f32, tag="w1c")
                        w2c = rs.tile([P, 1], f32, tag="w2c")
                        nc.vector.tensor_mul(w1c[:], m8[:, 0:1], rd[:])
                        nc.vector.tensor_mul(w2c[:], m8[:, 1:2], rd[:])
                        # combine tile over all experts:
                        #   eq1*w1 + eq2*(1-eq1)*w2   (tie-robust)
                        eq1 = rs.tile([P, E], f32, tag="eq1")
                        eq2 = rs.tile([P, E], f32, tag="eq2")
                        nc.vector.tensor_scalar(
                            out=eq1[:], in0=ex[:], scalar1=m8[:, 0:1],
                            scalar2=None, op0=mybir.AluOpType.is_equal)
                        nc.vector.tensor_scalar(
                            out=eq2[:], in0=ex[:], scalar1=m8[:, 1:2],
                            scalar2=None, op0=mybir.AluOpType.is_equal)
                        t2 = rs.tile([P, E], f32, tag="t2")
                        nc.vector.tensor_mul(t2[:], eq2[:], eq1[:])
                        nc.vector.tensor_sub(t2[:], eq2[:], t2[:])
                        ctile = rs.tile([P, E], f32, tag="ctile")
                        nc.vector.tensor_scalar_mul(ctile[:], eq1[:], w1c[:, 0:1])
                        nc.vector.scalar_tensor_tensor(
                            out=ctile[:], in0=t2[:], scalar=w2c[:, 0:1],
                            in1=ctile[:], op0=mybir.AluOpType.mult,
                            op1=mybir.AluOpType.add)
                        cnt_t = rs.tile([P, E], f32, tag="cnt_t")
                        nc.vector.tensor_scalar(
                            out=cnt_t[:], in0=ctile[:], scalar1=0.0,
                            scalar2=None, op0=mybir.AluOpType.is_gt)
                        # this core's expert column via esel
                        csel = rs.tile([P, E], f32, tag="csel")
                        nc.vector.tensor_mul(csel[:], ctile[:], esel_sb[:])
                        nc.vector.reduce_sum(
                            out=cmb_cols[:, sub:sub + 1], in_=csel[:],
                            axis=mybir.AxisListType.X)
                        nc.vector.tensor_scalar(
                            out=msk_cols[:, sub:sub + 1],
                            in0=cmb_cols[:, sub:sub + 1], scalar1=0.0,
                            scalar2=None, op0=mybir.AluOpType.is_gt)
                        # aux accumulators (all experts)
                        nc.tensor.matmul(probs_sum_ps[:], lhsT=probs[:],
                                         rhs=ones_col[:], start=(sub == 0),
                                         stop=(sub == NTT - 1))
                        nc.tensor.matmul(cnt_sum_ps[:], lhsT=cnt_t[:],
                                         rhs=ones_col[:], start=(sub == 0),
                                         stop=(sub == NTT - 1))

                aux_sb = rp.tile([E, 2], f32)
                nc.vector.tensor_copy(aux_sb[:, 0:1], probs_sum_ps[:])
                nc.vector.tensor_copy(aux_sb[:, 1:2], cnt_sum_ps[:])
                nc.sync.dma_start(out=aux_out[:, :], in_=aux_sb[:])

            # ---------------- Phase C: compaction slots ----------------
            with tc.tile_pool(name="cmp", bufs=1) as cm, \
                 tc.tile_pool(name="psC", bufs=1, space="PSUM") as psC:
                zero_row = cm.tile([1, NTT], f32)
                nc.vector.memset(zero_row[:], 0.0)

                def excl_base(mask_ap, offset, tagp):
                    cnt_ps = psC.tile([1, NTT], f32, space="PSUM", tag=f"cnt{tagp}")
                    nc.tensor.matmul(cnt_ps[:], lhsT=ones_col[:], rhs=mask_ap,
                                     start=True, stop=True)
                    cnt_sb = cm.tile([1, NTT], f32, tag=f"cntsb{tagp}")
                    nc.vector.tensor_copy(cnt_sb[:], cnt_ps[:])
                    incl = cm.tile([1, NTT], f32, tag=f"incl{tagp}")
                    nc.vector.tensor_tensor_scan(
                        out=incl[:], data0=cnt_sb[:], data1=zero_row[:],
                        initial=0.0, op0=mybir.AluOpType.add,
                        op1=mybir.AluOpType.add)
                    base = cm.tile([1, NTT], f32, tag=f"base{tagp}")
                    nc.vector.tensor_sub(base[:], incl[:], cnt_sb[:])
                    if offset:
                        nc.vector.tensor_scalar_add(base[:], base[:], float(offset))
                    return base

                base_a = excl_base(msk_cols[:], 0, "a")
                pos_a = psC.tile([P, NTT], f32, space="PSUM", tag="posa")
                nc.tensor.matmul(pos_a[:], lhsT=ustrict[:], rhs=msk_cols[:],
                                 start=True, stop=False)
                nc.tensor.matmul(pos_a[:], lhsT=ones_row1[:], rhs=base_a[:],
                                 start=False, stop=True)

                msk_u = cm.tile([P, NTT], f32)
                nc.vector.tensor_scalar(
                    out=msk_u[:], in0=msk_cols[:], scalar1=-1.0, scalar2=1.0,
                    op0=mybir.AluOpType.mult, op1=mybir.AluOpType.add)
                base_u = excl_base(msk_u[:], C, "u")
                pos_u = psC.tile([P, NTT], f32, space="PSUM", tag="posu")
                nc.tensor.matmul(pos_u[:], lhsT=ustrict[:], rhs=msk_u[:],
                                 start=True, stop=False)
                nc.tensor.matmul(pos_u[:], lhsT=ones_row1[:], rhs=base_u[:],
                                 start=False, stop=True)

                msk_u8 = cm.tile([P, NTT], mybir.dt.uint8)
                nc.vector.tensor_copy(msk_u8[:], msk_cols[:])
                slot_f = rp.tile([P, NTT], f32)
                nc.vector.tensor_copy(slot_f[:], pos_u[:])
                if DEBUG:
                    pa_sb = cm.tile([P, NTT], f32, tag="pa_sb")
                    nc.vector.tensor_copy(pa_sb[:], pos_a[:])
                    nc.sync.dma_start(out=dbg_posa[:, :], in_=pa_sb[:])
                    pu_sb = cm.tile([P, NTT], f32, tag="pu_sb")
                    nc.vector.tensor_copy(pu_sb[:], pos_u[:])
                    nc.sync.dma_start(out=dbg_posu[:, :], in_=pu_sb[:])
                    nc.sync.dma_start(out=dbg_msku8[:, :], in_=msk_u8[:])
                nc.vector.copy_predicated(slot_f[:], msk_u8[:], pos_a[:])
                nc.vector.tensor_scalar_min(slot_f[:], slot_f[:], float(XPN - 1))
                slot_i = rp.tile([P, NTT], i32)
                nc.vector.tensor_copy(slot_i[:], slot_f[:])
                if DEBUG:
                    nc.sync.dma_start(out=dbg_slotf[:, :], in_=slot_f[:])
                    nc.sync.dma_start(out=dbg_slot2[:, :], in_=slot_i[:])

            if DEBUG:
                nc.sync.dma_start(out=dbg_ids[:, :], in_=ids_f32[:])
                nc.sync.dma_start(out=dbg_slot[:, :], in_=slot_i[:])
                nc.sync.dma_start(out=dbg_msk[:, :], in_=msk_cols[:])
                nc.sync.dma_start(out=dbg_cmb[:, :], in_=cmb_cols[:])

            # ---------------- Phase S: permute-scatter ----------------
            with tc.tile_pool(name="scat", bufs=4) as sc:
                zt = sc.tile([P, W], f32, tag="zt")
                nc.vector.memset(zt[:], 0.0)
                for s in range(NS):
                    nc.sync.dma_start(out=xp[s * P:(s + 1) * P, :], in_=zt[:])
                for ti in range(NTT):
                    st = sc.tile([P, W], f32, tag="st")
                    nc.sync.dma_start(out=st[:, 0:H],
                                      in_=x_tm[ti * P:(ti + 1) * P, :])
                    nc.vector.tensor_copy(st[:, H:H + 1], cmb_cols[:, ti:ti + 1])
                    nc.vector.tensor_copy(st[:, H + 1:H + 2], ids_f32[:, ti:ti + 1])
                    nc.gpsimd.indirect_dma_start(
                        out=xp[:, :],
                        out_offset=bass.IndirectOffsetOnAxis(
                            ap=slot_i[:, ti:ti + 1], axis=0),
                        in_=st[:, :], in_offset=None)

            # ---------------- Phase T + F1 ----------------
            with tc.tile_pool(name="xgts", bufs=1) as xg, \
                 tc.tile_pool(name="tld", bufs=3) as tl, \
                 tc.tile_pool(name="psT", bufs=2, space="PSUM") as psT:
                xgTs = xg.tile([P, KH * C], ffn_store)
                cmb_slot = rp.tile([P, NS], f32)
                idsl = tl.tile([P, NS], f32, tag="idsl_all")
                for s in range(0 if ABLATE >= 3 else NS):
                    ld = tl.tile([P, W], f32, tag="ld")
                    nc.sync.dma_start(out=ld[:], in_=xp[s * P:(s + 1) * P, :])
                    nc.vector.tensor_copy(cmb_slot[:, s:s + 1], ld[:, H:H + 1])
                    nc.vector.tensor_copy(idsl[:, s:s + 1], ld[:, H + 1:H + 2])
                    if DEBUG and s == 7:
                        nc.sync.dma_start(out=dbg_ld[:, :], in_=ld[:])
                    for k in range(KH):
                        pt = psT.tile([P, P], f32, space="PSUM", tag="pt")
                        nc.tensor.transpose(out=pt[:],
                                            in_=ld[:, k * P:(k + 1) * P],
                                            identity=ident[:])
                        nc.vector.tensor_copy(
                            xgTs[:, k * C + s * P: k * C + (s + 1) * P], pt[:])
                # slot->token table, partition-major: ids_out[p, s] is the
                # token id of slot s*128+p (host transposes).
                idt = tl.tile([P, NS], i32, tag="idt")
                nc.vector.tensor_copy(idt[:], idsl[:])
                ids_dma = nc.sync.dma_start(out=ids_out[:, :], in_=idt[:])

                # F1: h = gelu(x @ w1T + b1), stored transposed [f, slot]
                with tc.tile_pool(name="w1p", bufs=2) as w1p, \
                     tc.tile_pool(name="hact", bufs=3) as hp, \
                     tc.tile_pool(name="psF1", bufs=3, space="PSUM") as psF1:
                    for ft in range(0 if ABLATE >= 2 else KF):
                        w1s = w1p.tile([P, KH * P], ffn_store, tag="w1s")
                        w1_dma = nc.sync.dma_start(
                            out=w1s[:],
                            in_=w1R[:, ft * KH * P:(ft + 1) * KH * P])
                        if ft == 0:
                            # The f32r matmul path rewrites unrelated SBUF
                            # words (upper partitions) with f32r-rounded
                            # copies; get the exact token-id table out to
                            # DRAM before any F1 f32r matmul can run.
                            add_dep_helper(
                                w1_dma.ins, ids_dma.ins, sync=True,
                                reason="flush ids before f32r FFN")
                        for g in range(NG):
                            ph = psF1.tile([P, 512], f32, space="PSUM", tag="ph")
                            for k in range(KH):
                                nc.tensor.matmul(
                                    ph[:],
                                    lhsT=w1s[:, k * P:(k + 1) * P],
                                    rhs=xgTs[:, k * C + g * 512:
                                             k * C + (g + 1) * 512],
                                    start=(k == 0), stop=(k == KH - 1))
                            ha = hp.tile([P, 512], ffn_store, tag="ha")
                            nc.scalar.activation(
                                ha[:], ph[:], mybir.ActivationFunctionType.Gelu,
                                bias=b1_sb[:, ft:ft + 1], scale=1.0)
                            nc.sync.dma_start(
                                out=h_dram[:, ft * C + g * 512:
                                           ft * C + (g + 1) * 512],
                                in_=ha[:])

            # ---------------- Phase F2 ----------------
            with tc.tile_pool(name="w2p", bufs=1) as w2p, \
                 tc.tile_pool(name="hsp", bufs=2) as hsp, \
                 tc.tile_pool(name="yst", bufs=3) as yp, \
                 tc.tile_pool(name="psF2", bufs=4, space="PSUM") as psF2:
                w2_sb = w2p.tile([P, KF * H], ffn_store)
                for q in range(0 if ABLATE >= 1 else 8):
                    seg = KF * H // 8
                    nc.sync.dma_start(out=w2_sb[:, q * seg:(q + 1) * seg],
                                      in_=w2R[:, q * seg:(q + 1) * seg])
                h_view = h_dram[:, :].rearrange("p (k c) -> p k c", k=KF)
                for sg in range(0 if ABLATE >= 1 else NS // 2):
                    hs = hsp.tile([P, KF * 256], ffn_store, tag="hs")
                    # hs[:, k*256+j] = h_dram[:, k*C + sg*256 + j]
                    nc.sync.dma_start(
                        out=hs[:].rearrange("p (k j) -> p k j", k=KF),
                        in_=h_view[:, :, sg * 256:(sg + 1) * 256])
                    for s in range(2):
                        si = sg * 2 + s
                        for oc in range(2):
                            py = psF2.tile([P, 512], f32, space="PSUM", tag="py")
                            for k in range(KF):
                                nc.tensor.matmul(
                                    py[:],
                                    lhsT=hs[:, k * 256 + s * P:
                                         k * 256 + (s + 1) * P],
                                    rhs=w2_sb[:, k * H + oc * 512:
                                          k * H + (oc + 1) * 512],
                                    start=(k == 0), stop=False)
                            nc.tensor.matmul(
                                py[:], lhsT=ones_row1[:],
                                rhs=b2_sb[:, oc * 512:(oc + 1) * 512],
                                start=False, stop=True)
                            ysb = yp.tile([P, 512], f32, tag="ysb")
                            nc.vector.tensor_scalar_mul(
                                ysb[:], py[:], cmb_slot[:, si:si + 1])
                            nc.sync.dma_start(
                                out=yc[si * P:(si + 1) * P,
                                       oc * 512:(oc + 1) * 512],
                                in_=ysb[:])

    nc.compile()
    return nc


_prog_cache = {}


def _get_program():
    if "nc" not in _prog_cache:
        _prog_cache["nc"] = _build_program()
    return _prog_cache["nc"]


def _prep_inputs(x, router_w, fc1_w, fc1_b, fc2_w, fc2_b):
    """Host-side sharding/layout prep. Returns list of 8 in_maps."""
    xf = np.ascontiguousarray(np.asarray(x, np.float32).reshape(T, H))
    # xTR[p, (tb*KH+k)*512 + j] = xf[tb*512+j, k*128+p]
    xTR = np.ascontiguousarray(
        xf.T.reshape(KH, P, NTB, 512).transpose(1, 2, 0, 3).reshape(P, -1))
    rw = np.asarray(router_w, np.float32)
    rwR = np.ascontiguousarray(
        rw.T.reshape(KH, P, E).transpose(1, 0, 2).reshape(P, -1))
    esel = np.zeros((8, P, E), np.float32)
    for e in range(E):
        esel[e, :, e] = 1.0

    if FFN_DT == "float32r":
        wcast = _round_f32r
    elif FFN_DT == "bfloat16":
        import ml_dtypes
        wcast = lambda a: a.astype(ml_dtypes.bfloat16)
    else:
        wcast = lambda a: a
    in_maps = []
    for e in range(E):
        w1T = np.asarray(fc1_w, np.float32)[e].T        # [H, F]
        w1R = np.ascontiguousarray(
            w1T.reshape(KH, P, KF, P).transpose(1, 2, 0, 3).reshape(P, -1))
        w2T = np.asarray(fc2_w, np.float32)[e].T        # [F, H]
        w2R = np.ascontiguousarray(
            w2T.reshape(KF, P, H).transpose(1, 0, 2).reshape(P, -1))
        b1R = np.ascontiguousarray(
            np.asarray(fc1_b, np.float32)[e].reshape(KF, P).T)
        b2 = np.asarray(fc2_b, np.float32)[e:e + 1, :]
        in_maps.append({
            "xTR": xTR, "x_tm": xf, "rwR": rwR,
            "w1R": wcast(w1R), "b1R": b1R,
            "w2R": wcast(w2R), "b2": np.ascontiguousarray(b2),
            "esel": esel[e],
        })
    return in_maps


def kernel(x, router_w, fc1_w, fc1_b, fc2_w, fc2_b):
    nc = _get_program()
    in_maps = _prep_inputs(x, router_w, fc1_w, fc1_b, fc2_w, fc2_b)
    res = bass_utils.run_bass_kernel_spmd(
        nc, in_maps, core_ids=list(range(E)), trace=False)
    _prog_cache["last_res"] = res

    out = np.zeros((T, H), np.float32)
    for e in range(E):
        r = res.results[e]
        ycv = r["yc"]                       # [C, H]
        ids = r["ids_out"].T.reshape(-1)    # [C] slot-order
        ne = int(round(float(r["aux_out"][e, 1])))
        ne = min(ne, C)
        sel = ids[:ne]
        out[sel] += ycv[:ne]

    aux = res.results[0]["aux_out"]
    sum_probs = aux[:, 0].astype(np.float64)
    counts = aux[:, 1].astype(np.float64)
    freq = counts / (T * 2)
    aux_loss = np.float32(0.01 * E * np.sum((sum_probs / T) * freq))

    B, S = 4, 2048
    return out.reshape(B, S, H), aux_loss


# revision 15
# speedup vs baseline: 24484.6013x; 24484.6013x over previous
"""MoE layer (8 experts, top-2) on 8 TRN2 NeuronCores — expert-parallel.

Each core c owns expert e=c. Per core:
  R: route ALL T=8192 tokens (fp32 router matmuls + softmax + max8 top-2).
  C: compute compaction slots with triangular-matmul prefix sums.
  S: indirect-scatter tiny (combine, token-id) rows into a slot-indexed
     table (assigned tokens -> slots [0,C), rest -> dump area).
  T: per slot tile, indirect-GATHER the assigned token rows from x,
     PE-transpose them into [H-chunk, slot] layout for the FFN.
  F1/F2: fc1 -> gelu -> fc2 in float32r (fp32 storage, 12-bit-mantissa
     matmul datapath, full-rate PE), scale rows by combine weight.
Host combines: out[ids_e] += yc_e per expert; aux loss from device sums.

Self-contained: hardcodes shapes for nn_MoELayer (B=4,S=2048,H=1024,
F=4096,E=8,K=2).

HW notes (measured on TRN2):
  - float32r = RNE to 11 explicit mantissa bits; 1 cyc/row at N>=256.
  - PE fp32 transpose goes through the f32r datapath (lossy >= 2^12) —
    fine for x data (rounded to f32r anyway), never used for token ids.
  - The f32r matmul path rewrites unrelated SBUF words (upper partitions)
    with f32r-rounded copies of themselves; all exact data (token ids)
    must be flushed to DRAM before F1 starts (add_dep_helper below).
"""

import numpy as np

import concourse.bacc as bacc
import concourse.bass as bass
import concourse.mybir as mybir
import concourse.tile as tile
from concourse import bass_utils
from concourse.masks import make_identity, make_upper_triangular
from concourse.tile import add_dep_helper

P = 128
T = 8192          # tokens
H = 1024          # hidden
F = 4096          # ffn
E = 8             # experts
C = 2560          # per-expert capacity (observed max load 2175)
XPN = C + T       # slot table rows (assigned + dump area)
NTT = T // P      # 64 token tiles
NTB = T // 512    # 16 router blocks
NS = C // P       # 20 slot tiles
NG = C // 512     # 5 fc1 groups of 512 slots
KH = H // P       # 8 contraction chunks over H
KF = F // P       # 32 contraction chunks over F

# FFN matmul input dtype: "float32" (exact, 4 cyc/row) or "float32r"
# (fp32 storage, ~1e-4 rel err, 1 cyc/row at N>=256).
FFN_DT = "float32r"
ABLATE = 0   # bench: 5=R+C only, 4=+zerofill, 3=+scatter, 2=+gatherT, 1=+F1, 0=full

f32 = mybir.dt.float32
i32 = mybir.dt.int32


def _round_f32r(a):
    """Replicate HW float32r rounding: RNE to 11 explicit mantissa bits."""
    x = np.ascontiguousarray(a, np.float32).view(np.uint32).astype(np.uint64)
    shift = 23 - 11
    add = (1 << (shift - 1)) - 1 + ((x >> shift) & 1)
    return ((x + add) >> shift << shift).astype(np.uint32).view(np.float32)


def _build_program():
    nc = bacc.Bacc("TRN2", target_bir_lowering=False, debug=False)

    ffn_store = mybir.dt.float32r if FFN_DT == "float32r" else f32

    # ---- I/O ----
    xTR = nc.dram_tensor("xTR", [P, NTB * KH * 512], f32, kind="ExternalInput")
    x_tm = nc.dram_tensor("x_tm", [T, H], f32, kind="ExternalInput")
    rwR = nc.dram_tensor("rwR", [P, KH * E], f32, kind="ExternalInput")
    w1R = nc.dram_tensor("w1R", [P, KF * KH * P], ffn_store, kind="ExternalInput")
    b1R = nc.dram_tensor("b1R", [P, KF], f32, kind="ExternalInput")
    w2R = nc.dram_tensor("w2R", [P, KF * H], ffn_store, kind="ExternalInput")
    b2 = nc.dram_tensor("b2", [1, H], f32, kind="ExternalInput")
    esel = nc.dram_tensor("esel", [P, E], f32, kind="ExternalInput")

    yc = nc.dram_tensor("yc", [C, H], f32, kind="ExternalOutput")
    ids_out = nc.dram_tensor("ids_out", [P, NS], i32, kind="ExternalOutput")
    aux_out = nc.dram_tensor("aux_out", [E, 2], f32, kind="ExternalOutput")

    # ---- internal DRAM ----
    idc = nc.dram_tensor("idc", [XPN, 2], f32)       # (combine, token-id)
    h_dram = nc.dram_tensor("h_dram", [P, KF * C], ffn_store)

    with tile.TileContext(nc) as tc:
        with tc.tile_pool(name="consts", bufs=1) as cp, \
             tc.tile_pool(name="route", bufs=1) as rp:

            ident = cp.tile([P, P], f32)
            make_identity(nc, ident[:])
            ustrict = cp.tile([P, P], f32)
            make_upper_triangular(nc, ustrict[:], val=1.0, diag=False)
            ones_col = cp.tile([P, 1], f32)
            nc.vector.memset(ones_col[:], 1.0)
            ones_row1 = cp.tile([1, P], f32)
            nc.vector.memset(ones_row1[:], 1.0)
            rw_sb = cp.tile([P, KH * E], f32)
            nc.sync.dma_start(out=rw_sb[:], in_=rwR[:, :])
            esel_sb = cp.tile([P, E], f32)
            nc.sync.dma_start(out=esel_sb[:], in_=esel[:, :])
            b1_sb = cp.tile([P, KF], f32)
            nc.sync.dma_start(out=b1_sb[:], in_=b1R[:, :])
            b2_sb = cp.tile([1, H], f32)
            nc.sync.dma_start(out=b2_sb[:], in_=b2[:, :])

            # persistent routing results
            cmb_cols = rp.tile([P, NTT], f32)
            msk_cols = rp.tile([P, NTT], f32)
            ids_i32 = rp.tile([P, NTT], i32)
            nc.gpsimd.iota(ids_i32[:], pattern=[[P, NTT]], base=0,
                           channel_multiplier=1)
            ids_f32 = rp.tile([P, NTT], f32)
            nc.vector.tensor_copy(ids_f32[:], ids_i32[:])

            # ---------------- Phase R: routing ----------------
            with tc.tile_pool(name="psA", bufs=1, space="PSUM") as psA, \
                 tc.tile_pool(name="rblk", bufs=2) as rb, \
                 tc.tile_pool(name="rsm", bufs=3) as rs, \
                 tc.tile_pool(name="psR", bufs=3, space="PSUM") as psR:
                probs_sum_ps = psA.tile([E, 1], f32, space="PSUM")
                cnt_sum_ps = psA.tile([E, 1], f32, space="PSUM")
                for tb in range(NTB):
                    xTb = rb.tile([P, KH * 512], f32, tag="xTb")
                    nc.sync.dma_start(
                        out=xTb[:], in_=xTR[:, tb * KH * 512:(tb + 1) * KH * 512])
                    for m in range(4):
                        sub = tb * 4 + m
                        lg = psR.tile([P, E], f32, space="PSUM", tag="lg")
                        for k in range(KH):
                            nc.tensor.matmul(
                                lg[:],
                                lhsT=xTb[:, k * 512 + m * P: k * 512 + (m + 1) * P],
                                rhs=rw_sb[:, k * E:(k + 1) * E],
                                start=(k == 0), stop=(k == KH - 1))
                        mx = rs.tile([P, 1], f32, tag="mx")
                        nc.vector.reduce_max(out=mx[:], in_=lg[:],
                                             axis=mybir.AxisListType.X)
                        mneg = rs.tile([P, 1], f32, tag="mneg")
                        nc.vector.tensor_scalar_mul(mneg[:], mx[:], -1.0)
                        ex = rs.tile([P, E], f32, tag="ex")
                        nc.scalar.activation(ex[:], lg[:],
                                             mybir.ActivationFunctionType.Exp,
                                             bias=mneg[:, 0:1], scale=1.0)
                        s = rs.tile([P, 1], f32, tag="s")
                        nc.vector.reduce_sum(out=s[:], in_=ex[:],
                                             axis=mybir.AxisListType.X)
                        r = rs.tile([P, 1], f32, tag="r")
                        nc.vector.reciprocal(r[:], s[:])
                        probs = rs.tile([P, E], f32, tag="probs")
                        nc.vector.tensor_scalar_mul(probs[:], ex[:], r[:, 0:1])
                        m8 = rs.tile([P, 8], f32, tag="m8")
                        nc.vector.max(out=m8[:], in_=ex[:])
                        den = rs.tile([P, 1], f32, tag="den")
                        nc.vector.tensor_add(den[:], m8[:, 0:1], m8[:, 1:2])
                        rd = rs.tile([P, 1], f32, tag="rd")
                        nc.vector.reciprocal(rd[:], den[:])
                        w1c = rs.tile([P, 1], f32, tag="w1c")
                        w2c = rs.tile([P, 1], f32, tag="w2c")
                        nc.vector.tensor_mul(w1c[:], m8[:, 0:1], rd[:])
                        nc.vector.tensor_mul(w2c[:], m8[:, 1:2], rd[:])
                        # combine = eq1*w1 + eq2*(1-eq1)*w2  (tie-robust)
                        eq1 = rs.tile([P, E], f32, tag="eq1")
                        eq2 = rs.tile([P, E], f32, tag="eq2")
                        nc.vector.tensor_scalar(
                            out=eq1[:], in0=ex[:], scalar1=m8[:, 0:1],
                            scalar2=None, op0=mybir.AluOpType.is_equal)
                        nc.vector.tensor_scalar(
                            out=eq2[:], in0=ex[:], scalar1=m8[:, 1:2],
                            scalar2=None, op0=mybir.AluOpType.is_equal)
                        t2 = rs.tile([P, E], f32, tag="t2")
                        nc.vector.tensor_mul(t2[:], eq2[:], eq1[:])
                        nc.vector.tensor_sub(t2[:], eq2[:], t2[:])
                        ctile = rs.tile([P, E], f32, tag="ctile")
                        nc.vector.tensor_scalar_mul(ctile[:], eq1[:], w1c[:, 0:1])
                        nc.vector.scalar_tensor_tensor(
                            out=ctile[:], in0=t2[:], scalar=w2c[:, 0:1],
                            in1=ctile[:], op0=mybir.AluOpType.mult,
                            op1=mybir.AluOpType.add)
                        cnt_t = rs.tile([P, E], f32, tag="cnt_t")
                        nc.vector.tensor_scalar(
                            out=cnt_t[:], in0=ctile[:], scalar1=0.0,
                            scalar2=None, op0=mybir.AluOpType.is_gt)
                        csel = rs.tile([P, E], f32, tag="csel")
                        nc.vector.tensor_mul(csel[:], ctile[:], esel_sb[:])
                        nc.vector.reduce_sum(
                            out=cmb_cols[:, sub:sub + 1], in_=csel[:],
                            axis=mybir.AxisListType.X)
                        nc.vector.tensor_scalar(
                            out=msk_cols[:, sub:sub + 1],
                            in0=cmb_cols[:, sub:sub + 1], scalar1=0.0,
                            scalar2=None, op0=mybir.AluOpType.is_gt)
                        nc.tensor.matmul(probs_sum_ps[:], lhsT=probs[:],
                                         rhs=ones_col[:], start=(sub == 0),
                                         stop=(sub == NTT - 1))
                        nc.tensor.matmul(cnt_sum_ps[:], lhsT=cnt_t[:],
                                         rhs=ones_col[:], start=(sub == 0),
                                         stop=(sub == NTT - 1))

                aux_sb = rp.tile([E, 2], f32)
                nc.vector.tensor_copy(aux_sb[:, 0:1], probs_sum_ps[:])
                nc.vector.tensor_copy(aux_sb[:, 1:2], cnt_sum_ps[:])
                aux_dma = nc.sync.dma_start(out=aux_out[:, :], in_=aux_sb[:])

            # ---------------- Phase C: compaction slots ----------------
            with tc.tile_pool(name="cmp", bufs=1) as cm, \
                 tc.tile_pool(name="psC", bufs=1, space="PSUM") as psC:
                zero_row = cm.tile([1, NTT], f32)
                nc.vector.memset(zero_row[:], 0.0)

                def excl_base(mask_ap, offset, tagp):
                    cnt_ps = psC.tile([1, NTT], f32, space="PSUM", tag=f"cnt{tagp}")
                    nc.tensor.matmul(cnt_ps[:], lhsT=ones_col[:], rhs=mask_ap,
                                     start=True, stop=True)
                    cnt_sb = cm.tile([1, NTT], f32, tag=f"cntsb{tagp}")
                    nc.vector.tensor_copy(cnt_sb[:], cnt_ps[:])
                    incl = cm.tile([1, NTT], f32, tag=f"incl{tagp}")
                    nc.vector.tensor_tensor_scan(
                        out=incl[:], data0=cnt_sb[:], data1=zero_row[:],
                        initial=0.0, op0=mybir.AluOpType.add,
                        op1=mybir.AluOpType.add)
                    base = cm.tile([1, NTT], f32, tag=f"base{tagp}")
                    nc.vector.tensor_sub(base[:], incl[:], cnt_sb[:])
                    if offset:
                        nc.vector.tensor_scalar_add(base[:], base[:], float(offset))
                    return base

                base_a = excl_base(msk_cols[:], 0, "a")
                pos_a = psC.tile([P, NTT], f32, space="PSUM", tag="posa")
                nc.tensor.matmul(pos_a[:], lhsT=ustrict[:], rhs=msk_cols[:],
                                 start=True, stop=False)
                nc.tensor.matmul(pos_a[:], lhsT=ones_row1[:], rhs=base_a[:],
                                 start=False, stop=True)

                msk_u = cm.tile([P, NTT], f32)
                nc.vector.tensor_scalar(
                    out=msk_u[:], in0=msk_cols[:], scalar1=-1.0, scalar2=1.0,
                    op0=mybir.AluOpType.mult, op1=mybir.AluOpType.add)
                base_u = excl_base(msk_u[:], C, "u")
                pos_u = psC.tile([P, NTT], f32, space="PSUM", tag="posu")
                nc.tensor.matmul(pos_u[:], lhsT=ustrict[:], rhs=msk_u[:],
                                 start=True, stop=False)
                nc.tensor.matmul(pos_u[:], lhsT=ones_row1[:], rhs=base_u[:],
                                 start=False, stop=True)

                msk_u8 = cm.tile([P, NTT], mybir.dt.uint8)
                nc.vector.tensor_copy(msk_u8[:], msk_cols[:])
                slot_f = rp.tile([P, NTT], f32)
                nc.vector.tensor_copy(slot_f[:], pos_u[:])
                nc.vector.copy_predicated(slot_f[:], msk_u8[:], pos_a[:])
                nc.vector.tensor_scalar_min(slot_f[:], slot_f[:], float(XPN - 1))
                slot_i = rp.tile([P, NTT], i32)
                nc.vector.tensor_copy(slot_i[:], slot_f[:])

            # ------------- Phase S: scatter (combine, id) pairs -------------
            with tc.tile_pool(name="scat", bufs=4) as sc:
                zt = sc.tile([P, 2 * C // P], f32, tag="zt")
                nc.vector.memset(zt[:], 0.0)
                # idc rows [0:C) are contiguous: view as [P, 2C/P]
                nc.sync.dma_start(
                    out=idc[0:C, :].rearrange("(a b) c -> a (b c)", a=P),
                    in_=zt[:])
                for ti in range(0 if ABLATE >= 4 else NTT):
                    st = sc.tile([P, 2], f32, tag="st")
                    nc.vector.tensor_copy(st[:, 0:1], cmb_cols[:, ti:ti + 1])
                    nc.vector.tensor_copy(st[:, 1:2], ids_f32[:, ti:ti + 1])
                    nc.gpsimd.indirect_dma_start(
                        out=idc[:, :],
                        out_offset=bass.IndirectOffsetOnAxis(
                            ap=slot_i[:, ti:ti + 1], axis=0),
                        in_=st[:, :], in_offset=None)

            # --------- Phase T: gather + transpose assigned tokens ---------
            w2a_pool = tc.tile_pool(name="w2a", bufs=1)
            w2a = w2a_pool.__enter__().tile([P, KF * H // 2], ffn_store)
            for q in range(0 if ABLATE >= 1 else 4):
                seg = KF * H // 8
                nc.sync.dma_start(out=w2a[:, q * seg:(q + 1) * seg],
                                  in_=w2R[:, q * seg:(q + 1) * seg])
            with tc.tile_pool(name="xgts", bufs=1) as xg, \
                 tc.tile_pool(name="tld", bufs=3) as tl, \
                 tc.tile_pool(name="psT", bufs=2, space="PSUM") as psT:
                xgTs = xg.tile([P, KH * C], ffn_store)
                cmb_slot = rp.tile([P, NS], f32)
                idsl = tl.tile([P, NS], f32, tag="idsl_all")
                for s in range(0 if ABLATE >= 3 else NS):
                    idcl = tl.tile([P, 2], f32, tag="idcl")
                    nc.sync.dma_start(out=idcl[:], in_=idc[s * P:(s + 1) * P, :])
                    nc.vector.tensor_copy(cmb_slot[:, s:s + 1], idcl[:, 0:1])
                    nc.vector.tensor_copy(idsl[:, s:s + 1], idcl[:, 1:2])
                    gix = tl.tile([P, 1], i32, tag="gix")
                    nc.vector.tensor_copy(gix[:], idcl[:, 1:2])
                    ldx = tl.tile([P, H], f32, tag="ldx")
                    nc.gpsimd.indirect_dma_start(
                        out=ldx[:, :], out_offset=None,
                        in_=x_tm[:, :],
                        in_offset=bass.IndirectOffsetOnAxis(ap=gix[:, 0:1], axis=0))
                    for k in range(KH):
                        pt = psT.tile([P, P], f32, space="PSUM", tag="pt")
                        nc.tensor.transpose(out=pt[:],
                                            in_=ldx[:, k * P:(k + 1) * P],
                                            identity=ident[:])
                        nc.vector.tensor_copy(
                            xgTs[:, k * C + s * P: k * C + (s + 1) * P], pt[:])
                if ABLATE < 3:
                    # slot->token table, partition-major (host transposes);
                    # never goes through the PE (f32r-lossy for ids >= 4096).
                    idt = tl.tile([P, NS], i32, tag="idt")
                    nc.vector.tensor_copy(idt[:], idsl[:])
                    ids_dma = nc.sync.dma_start(out=ids_out[:, :], in_=idt[:])

                # ---------------- Phase F1 ----------------
                with tc.tile_pool(name="w1p", bufs=2) as w1p, \
                     tc.tile_pool(name="hact", bufs=3) as hp, \
                     tc.tile_pool(name="psF1", bufs=3, space="PSUM") as psF1:
                    for ft in range(0 if ABLATE >= 2 else KF):
                        w1s = w1p.tile([P, KH * P], ffn_store, tag="w1s")
                        w1_dma = nc.sync.dma_start(
                            out=w1s[:],
                            in_=w1R[:, ft * KH * P:(ft + 1) * KH * P])
                        if ft == 0:
                            # exact ids must be in DRAM before any f32r MM
                            # (see HW notes in module docstring)
                            add_dep_helper(
                                w1_dma.ins, ids_dma.ins, sync=True,
                                reason="flush ids before f32r FFN")
                            add_dep_helper(
                                w1_dma.ins, aux_dma.ins, sync=True,
                                reason="flush aux before f32r FFN")
                        for g in range(NG):
                            ph = psF1.tile([P, 512], f32, space="PSUM", tag="ph")
                            for k in range(KH):
                                nc.tensor.matmul(
                                    ph[:],
                                    lhsT=w1s[:, k * P:(k + 1) * P],
                                    rhs=xgTs[:, k * C + g * 512:
                                             k * C + (g + 1) * 512],
                                    start=(k == 0), stop=(k == KH - 1))
                            ha = hp.tile([P, 512], ffn_store, tag="ha")
                            nc.scalar.activation(
                                ha[:], ph[:], mybir.ActivationFunctionType.Gelu,
                                bias=b1_sb[:, ft:ft + 1], scale=1.0)
                            nc.sync.dma_start(
                                out=h_dram[:, ft * C + g * 512:
                                           ft * C + (g + 1) * 512],
                                in_=ha[:])

            # ---------------- Phase F2 ----------------
            with tc.tile_pool(name="w2p", bufs=1) as w2p, \
                 tc.tile_pool(name="hsp", bufs=2) as hsp, \
                 tc.tile_pool(name="yst", bufs=3) as yp, \
                 tc.tile_pool(name="psF2", bufs=4, space="PSUM") as psF2:
                w2b = w2p.tile([P, KF * H // 2], ffn_store)
                half = KF * H // 2
                for q in range(0 if ABLATE >= 1 else 4):
                    seg = KF * H // 8
                    nc.sync.dma_start(out=w2b[:, q * seg:(q + 1) * seg],
                                      in_=w2R[:, half + q * seg:half + (q + 1) * seg])

                def w2_slice(k, oc, n0, n1):
                    # w2R columns k*H + oc*512 + [n0:n1]; halves split at KF*H/2
                    col = k * H + oc * 512
                    if col < half:
                        return w2a[:, col + n0:col + n1]
                    return w2b[:, col - half + n0:col - half + n1]
                h_view = h_dram[:, :].rearrange("p (k c) -> p k c", k=KF)
                for sg in range(0 if ABLATE >= 1 else NS // 2):
                    hs = hsp.tile([P, KF * 256], ffn_store, tag="hs")
                    nc.sync.dma_start(
                        out=hs[:].rearrange("p (k j) -> p k j", k=KF),
                        in_=h_view[:, :, sg * 256:(sg + 1) * 256])
                    for s in range(2):
                        si = sg * 2 + s
                        for oc in range(2):
                            py = psF2.tile([P, 512], f32, space="PSUM", tag="py")
                            for k in range(KF):
                                nc.tensor.matmul(
                                    py[:],
                                    lhsT=hs[:, k * 256 + s * P:
                                            k * 256 + (s + 1) * P],
                                    rhs=w2_slice(k, oc, 0, 512),
                                    start=(k == 0), stop=False)
                            nc.tensor.matmul(
                                py[:], lhsT=ones_row1[:],
                                rhs=b2_sb[:, oc * 512:(oc + 1) * 512],
                                start=False, stop=True)
                            ysb = yp.tile([P, 512], f32, tag="ysb")
                            nc.vector.tensor_scalar_mul(
                                ysb[:], py[:], cmb_slot[:, si:si + 1])
                            nc.sync.dma_start(
                                out=yc[si * P:(si + 1) * P,
                                       oc * 512:(oc + 1) * 512],
                                in_=ysb[:])

            w2a_pool.__exit__(None, None, None)

    nc.compile()
    return nc


_prog_cache = {}


def _get_program():
    if "nc" not in _prog_cache:
        _prog_cache["nc"] = _build_program()
    return _prog_cache["nc"]


def _prep_inputs(x, router_w, fc1_w, fc1_b, fc2_w, fc2_b):
    """Host-side sharding/layout prep. Returns list of 8 in_maps."""
    xf = np.ascontiguousarray(np.asarray(x, np.float32).reshape(T, H))
    xTR = np.ascontiguousarray(
        xf.T.reshape(KH, P, NTB, 512).transpose(1, 2, 0, 3).reshape(P, -1))
    rw = np.asarray(router_w, np.float32)
    rwR = np.ascontiguousarray(
        rw.T.reshape(KH, P, E).transpose(1, 0, 2).reshape(P, -1))
    esel = np.zeros((8, P, E), np.float32)
    for e in range(E):
        esel[e, :, e] = 1.0

    wcast = _round_f32r if FFN_DT == "float32r" else (lambda a: a)
    in_maps = []
    for e in range(E):
        w1T = np.asarray(fc1_w, np.float32)[e].T        # [H, F]
        w1R = np.ascontiguousarray(
            w1T.reshape(KH, P, KF, P).transpose(1, 2, 0, 3).reshape(P, -1))
        w2T = np.asarray(fc2_w, np.float32)[e].T        # [F, H]
        w2R = np.ascontiguousarray(
            w2T.reshape(KF, P, H).transpose(1, 0, 2).reshape(P, -1))
        b1R = np.ascontiguousarray(
            np.asarray(fc1_b, np.float32)[e].reshape(KF, P).T)
        b2 = np.asarray(fc2_b, np.float32)[e:e + 1, :]
        in_maps.append({
            "xTR": xTR, "x_tm": xf, "rwR": rwR,
            "w1R": wcast(w1R), "b1R": b1R,
            "w2R": wcast(w2R), "b2": np.ascontiguousarray(b2),
            "esel": esel[e],
        })
    return in_maps


def kernel(x, router_w, fc1_w, fc1_b, fc2_w, fc2_b):
    nc = _get_program()
    in_maps = _prep_inputs(x, router_w, fc1_w, fc1_b, fc2_w, fc2_b)
    res = bass_utils.run_bass_kernel_spmd(
        nc, in_maps, core_ids=list(range(E)), trace=False)
    _prog_cache["last_res"] = res

    out = np.zeros((T, H), np.float32)
    for e in range(E):
        r = res.results[e]
        ycv = r["yc"]                       # [C, H]
        ids = r["ids_out"].T.reshape(-1)    # [C] slot-order
        ne = int(round(float(r["aux_out"][e, 1])))
        ne = min(ne, C)
        sel = ids[:ne]
        out[sel] += ycv[:ne]

    aux = res.results[0]["aux_out"]
    sum_probs = aux[:, 0].astype(np.float64)
    counts = aux[:, 1].astype(np.float64)
    freq = counts / (T * 2)
    aux_loss = np.float32(0.01 * E * np.sum((sum_probs / T) * freq))

    B, S = 4, 2048
    return out.reshape(B, S, H), aux_loss
